# revision 21
# baseline (speedup 1.0000x reference)
"""Trainium2 Bass kernel for BaseLayerWithLoRA:
    y = x @ W^T + b + (x @ lora_A^T) @ lora_B^T
  x [4,2048,4096] f32, W [4096,4096], b [4096], lora_A [16,4096], lora_B [4096,16]

Sharding: token-parallel across 8 cores (1024 tokens each, full O per core).
No collectives; LoRA is computed per-core on its own token slice.

Per-core device program (bf16 matmuls, fp32 PSUM accumulation):
  phase A (col-tiled 4x): arT partials land at partition offsets 0/32/64/96
    of one PSUM bank per 512-token half.  A zeroing matmul (all-zero
    stationary) opens each bank's accumulation group so the four strips can
    all accumulate with start=False (no whole-bank has_written races), and
    leaves exact 0.0 in the gap partitions.
  lora tail: ONE full 128-row matmul per (o-tile, half) against the
    UNREDUCED strip partials; lora_B^T is replicated at the four 32-row
    offsets inside the weight blob (zeros in gaps), so no cross-partition
    reduction is ever needed and the LDWEIGHTS stays a full-array load
    (prefetchable -- the K=16 row-group load stalled the PE ~190ns/tile).
  wave 1 (first 3 o-tiles): kc-outer over 6 PSUM tiles so the PE rides the
    incoming x-chunk DMAs with no head-of-line blocking; the first w blobs
    are DMA'd in 4 column pieces so kc=0 work starts early.
  waves 2+: o-tile-serial, kc-inner: 33 accumulating matmuls per (ot,half)
    into one PSUM bank; bias fused into the PSUM->SBUF eviction (DVE
    tensor_scalar_add with a per-partition scalar operand).
Host does data layout only (transposes / tiling / packing), no arithmetic.

Built on bacc.Bacc so compile() runs move_matmul_waits_to_ldweights and
generate_event_semaphores.
"""

import sys

if "/opt/trn_rl_repo" not in sys.path:
    sys.path.insert(0, "/opt/trn_rl_repo")

import numpy as np

B, S, I, O, R = 4, 2048, 4096, 4096, 16
NCORES = 8
NTOK = B * S                 # 8192 tokens
TPC = NTOK // NCORES         # 1024 tokens per core


def build_nc(tpc=TPC, i_dim=I, o_dim=O, r=R, tok_tile=512, mm_dtype="bfloat16",
             wave1_ots=3, w_bufs=5, ps_bufs=6, o_bufs=3):
    import concourse.bacc as bacc
    import concourse.mybir as mybir
    import concourse.tile as tile

    KC = i_dim // 128        # contraction chunks
    OT = o_dim // 128        # output-row tiles
    TT = tpc // tok_tile     # token tiles (halves)
    WF = KC * 128 + 128      # per-o-tile weight blob free size (W + lora part)
    NG = KC // 4             # phase-A chunk groups (4 strips each)
    W1 = wave1_ots
    f32 = mybir.dt.float32
    fmm = getattr(mybir.dt, mm_dtype)

    nc = bacc.Bacc("TRN2", target_bir_lowering=False, debug=False)
    xt = nc.declare_dram_parameter("xt", [KC, 128, tpc], fmm, isOutput=False)
    wt = nc.declare_dram_parameter("wt", [OT, 128, WF], fmm, isOutput=False)
    # wave-1 blobs again, pre-transposed to [128, W1, WF] so each column
    # piece is ONE DMA whose src/dst iteration orders agree
    wt1 = nc.declare_dram_parameter("wt1", [128, W1, WF], fmm, isOutput=False)
    at = nc.declare_dram_parameter("at", [128, KC, r], fmm, isOutput=False)
    bias = nc.declare_dram_parameter("bias", [128, OT], f32, isOutput=False)
    out = nc.declare_dram_parameter("out", [OT, 128, tpc], f32, isOutput=True)

    with tile.TileContext(nc) as tc:
        with (
            tc.tile_pool(name="const", bufs=1) as constp,
            tc.tile_pool(name="xpool", bufs=KC) as xpool,
            tc.tile_pool(name="wpool", bufs=w_bufs) as wpool,
            tc.tile_pool(name="opool", bufs=o_bufs) as opool,
            tc.tile_pool(name="psum", bufs=ps_bufs, space="PSUM") as psum_pool,
        ):
            zeros_sb = constp.tile([128, 512], fmm, name="zeros_sb")
            nc.gpsimd.memset(zeros_sb[:], 0)
            at_sb = constp.tile([128, KC, r], fmm, name="at_sb")
            arT_sb = constp.tile([128, tpc], fmm, name="arT_sb")

            # DMA issue order == completion order (single HW dynamic queue):
            # interleave x chunks with just-in-time wave-1 blob pieces so the
            # PE goes dense as early as possible and never starves.
            xts = [None] * KC

            def dma_x(kc):
                x_t = xpool.tile([128, tpc], fmm, tag="xchunk",
                                 name=f"xchunk{kc}")
                nc.sync.dma_start(x_t[:], xt[kc])
                xts[kc] = x_t

            # all W1 wave-1 blobs live in ONE tile so each column piece is a
            # single 3D DMA: every dma_start costs ~0.6us of serial Sync-
            # engine descriptor issue, so fewer+bigger transfers win the head
            w1_sb = wpool.tile([128, W1, WF], fmm, tag="w1all", bufs=1,
                               name="w1all")
            # wave-1 blob pieces: kc0-1 | kc2-7 | kc8-15 | kc16-23 | kc24-end
            pieces = [(0, 256), (256, 1024), (1024, 2048), (2048, 3072),
                      (3072, WF)]

            def dma_w1_piece(p):
                a, b_ = pieces[p]
                nc.sync.dma_start(w1_sb[:, :, a:b_], wt1[:, :, a:b_])

            dma_x(0)
            dma_w1_piece(0)
            dma_x(1)
            dma_w1_piece(1)
            dma_x(2)
            dma_x(3)
            dma_x(4)
            nc.sync.dma_start(at_sb[:], at[:])
            dma_x(5)
            dma_w1_piece(2)
            for kc in range(6, 14):
                dma_x(kc)
            dma_w1_piece(3)
            for kc in range(14, 22):
                dma_x(kc)
            dma_w1_piece(4)
            for kc in range(22, KC):
                dma_x(kc)
            b_sb = constp.tile([128, OT], f32, name="b_sb")
            nc.sync.dma_start(b_sb[:], bias[:])
            # prefetch whole blobs into the remaining fresh ring slots
            pre_sb = {}
            for i in range(W1, min(w_bufs, OT)):
                wsb = wpool.tile([128, WF], fmm, tag="wblob", name=f"wsb{i}")
                nc.sync.dma_start(wsb[:], wt[i])
                pre_sb[i] = wsb

            # phase-A PSUM banks; zero-MM opens the accumulation group and
            # writes exact zeros everywhere (incl. strip gap partitions)
            pa = [
                psum_pool.tile([128, tok_tile], f32, bufs=1, name=f"pa{h}")
                for h in range(TT)
            ]
            # zero-MMs depend only on the memset: they issue before any DMA
            # lands and open each bank's accumulation group with exact zeros
            for h in range(TT):
                nc.tensor.matmul(pa[h][:], zeros_sb[:, 0:128], zeros_sb[:],
                                 start=True, stop=False)
            # HAM warmups: accumulate +0 into the phase-A banks during the
            # otherwise-idle head so the PE clock gate opens (K=8/8) before
            # the first data matmul instead of ~3.4us after it
            for w in range(6):
                nc.tensor.matmul(pa[w % TT][:], zeros_sb[:, 0:128],
                                 zeros_sb[:], start=False, stop=False)

            # wave 1: kc-outer over W1 o-tiles riding the x DMA; each phase-A
            # strip block trails the main block of its chunk group so the
            # col-tiled strips stay adjacent (concurrent) in the PE queue
            ps1 = {}
            for i in range(W1):
                for h in range(TT):
                    ps1[(i, h)] = psum_pool.tile(
                        [128, tok_tile], f32, tag="psm", name=f"ps1_{i}_{h}")
            def main_block(g):
                for kc in range(4 * g, 4 * g + 4):
                    for h in range(TT):
                        for i in range(W1):
                            ts = slice(h * tok_tile, (h + 1) * tok_tile)
                            nc.tensor.matmul(
                                ps1[(i, h)][:],
                                w1_sb[:, i, kc * 128:(kc + 1) * 128],
                                xts[kc][:, ts],
                                start=(kc == 0),
                                stop=(kc == KC - 1),
                            )

            def pha_block(g):
                for h in range(TT):
                    ts = slice(h * tok_tile, (h + 1) * tok_tile)
                    for j in range(4):
                        kc = 4 * g + j
                        last = (g == NG - 1 and j == 3)
                        nc.tensor.matmul(
                            pa[h][32 * j:32 * j + r, :],
                            at_sb[:, kc, :],
                            xts[kc][:, ts],
                            start=False,
                            stop=last,
                            tile_position=(0, 32 * j),
                        )

            # groups 0..NG-3 ride the x DMA with their phase-A strips as
            # padding; main g6 streams while the DVE copies arT out; loras
            # before main g7 so the evictions trail the per-tile stop
            # matmuls (kc31)
            for g in range(NG - 2):
                main_block(g)
                pha_block(g)
            pha_block(NG - 2)
            pha_block(NG - 1)
            main_block(NG - 2)
            for h in range(TT):
                ts = slice(h * tok_tile, (h + 1) * tok_tile)
                nc.vector.tensor_copy(arT_sb[:, ts], pa[h][:])
            for h in range(TT):
                ts = slice(h * tok_tile, (h + 1) * tok_tile)
                for i in range(W1):
                    nc.tensor.matmul(
                        ps1[(i, h)][:],
                        w1_sb[:, i, KC * 128:KC * 128 + 128],
                        arT_sb[:, ts],
                        start=False,
                        stop=False,
                    )
            main_block(NG - 1)
            for h in range(TT):
                ts = slice(h * tok_tile, (h + 1) * tok_tile)
                for i in range(W1):
                    o_sb = opool.tile([128, tok_tile], f32, tag="osb",
                                      name=f"osb_w1_{i}_{h}")
                    nc.vector.tensor_scalar_add(o_sb[:], ps1[(i, h)][:],
                                                b_sb[:, i:i + 1])
                    nc.sync.dma_start(out[i, :, ts], o_sb[:])

            # waves 2+: o-tile-serial (proven 216ns/MM steady state)
            for ot in range(W1, OT):
                if ot in pre_sb:
                    w_sb = pre_sb[ot]
                else:
                    w_sb = wpool.tile([128, WF], fmm, tag="wblob",
                                      name=f"wsb{ot}")
                    nc.sync.dma_start(w_sb[:], wt[ot])
                for h in range(TT):
                    ts = slice(h * tok_tile, (h + 1) * tok_tile)
                    ps = psum_pool.tile([128, tok_tile], f32, tag="psm",
                                        name=f"ps_{ot}_{h}")
                    for kc in range(KC):
                        nc.tensor.matmul(
                            ps[:],
                            w_sb[:, kc * 128:(kc + 1) * 128],
                            xts[kc][:, ts],
                            start=(kc == 0),
                            stop=False,
                        )
                    nc.tensor.matmul(
                        ps[:],
                        w_sb[:, KC * 128:KC * 128 + 128],
                        arT_sb[:, ts],
                        start=False,
                        stop=True,
                    )
                    o_sb = opool.tile([128, tok_tile], f32, tag="osb",
                                      name=f"osb_{ot}_{h}")
                    if ot == OT - 1 and h == TT - 1:
                        # split the final eviction so the last output DMA
                        # starts as early as possible (shorter tail)
                        hw = tok_tile // 2
                        for q in range(2):
                            qs = slice(q * hw, (q + 1) * hw)
                            nc.vector.tensor_scalar_add(
                                o_sb[:, qs], ps[:, qs], b_sb[:, ot:ot + 1])
                            nc.sync.dma_start(
                                out[ot, :, h * tok_tile + q * hw:
                                    h * tok_tile + (q + 1) * hw],
                                o_sb[:, qs])
                    else:
                        nc.vector.tensor_scalar_add(o_sb[:], ps[:],
                                                    b_sb[:, ot:ot + 1])
                        nc.sync.dma_start(out[ot, :, ts], o_sb[:])
    nc.compile()
    return nc


def prep_inputs(x, W, b, lora_A, lora_B, tpc=TPC, ncores=NCORES,
                mm_dtype="bfloat16"):
    """Host-side layout marshalling (layout + dtype cast only)."""
    import ml_dtypes

    np_mm = np.float32 if mm_dtype == "float32r" else np.dtype(ml_dtypes.bfloat16)
    i_dim, o_dim, r = W.shape[1], W.shape[0], lora_A.shape[0]
    ntok = tpc * ncores
    x = np.ascontiguousarray(x, dtype=np.float32).reshape(ntok, i_dim)
    W = np.ascontiguousarray(W, dtype=np.float32)
    b = np.ascontiguousarray(b, dtype=np.float32)
    lora_A = np.ascontiguousarray(lora_A, dtype=np.float32)
    lora_B = np.ascontiguousarray(lora_B, dtype=np.float32)

    KC, OT = i_dim // 128, o_dim // 128
    WF = KC * 128 + 128
    # wt blob per o-tile: [ki, kc*128+oo] = W[ot*128+oo, kc*128+ki],
    # last 128 cols: lora_B^T slice replicated at row offsets 0/32/64/96
    # (gap rows zero) to match the unreduced phase-A strip partials.
    wtb = np.zeros((OT, 128, WF), dtype=np_mm)
    wtb[:, :, : KC * 128] = (
        W.reshape(OT, 128, KC, 128).transpose(0, 3, 2, 1).reshape(OT, 128, KC * 128)
    ).astype(np_mm)
    lbT = lora_B.reshape(OT, 128, r).transpose(0, 2, 1).astype(np_mm)
    for g in range(4):
        wtb[:, 32 * g:32 * g + r, KC * 128:] = lbT
    # wave-1 blobs pre-transposed for the combined piece DMAs
    wt1 = np.ascontiguousarray(wtb[0:3].transpose(1, 0, 2))
    # at[ki, kc, r] = lora_A[r, kc*128+ki]
    at = np.ascontiguousarray(
        lora_A.T.reshape(KC, 128, r).transpose(1, 0, 2).astype(np_mm)
    )
    # bias[p, ot] = b[ot*128+p]
    bias = np.ascontiguousarray(b.reshape(OT, 128).T)

    in_maps = []
    for c in range(ncores):
        xc = x[c * tpc : (c + 1) * tpc]  # [tpc, i_dim]
        # xt[kc, ki, t] = xc[t, kc*128+ki]
        xtc = np.ascontiguousarray(
            xc.reshape(tpc, KC, 128).transpose(1, 2, 0).astype(np_mm)
        )
        in_maps.append({"xt": xtc, "wt": wtb, "wt1": wt1, "at": at,
                        "bias": bias})
    return in_maps


def assemble_output(results):
    # each core: out[OT, 128, tpc] == y_c^T; tokens are block-sharded
    outT = np.concatenate([r["out"] for r in results], axis=2)  # [OT,128,ntok]
    o_dim = outT.shape[0] * 128
    ntok = outT.shape[2]
    y = outT.reshape(o_dim, ntok).T  # [ntok, o_dim]
    return np.ascontiguousarray(y)


def run(trace=False, trace_kwargs=None, mm_dtype="bfloat16", **inputs):
    from concourse.bass_utils import run_bass_kernel_spmd

    nc = build_nc(mm_dtype=mm_dtype)
    in_maps = prep_inputs(mm_dtype=mm_dtype, **inputs)
    res = run_bass_kernel_spmd(
        nc,
        in_maps,
        list(range(NCORES)),
        trace=trace,
        trace_kwargs=trace_kwargs or {},
    )
    return assemble_output(res.results).reshape(B, S, O), res


def kernel(**inputs):
    y, _ = run(trace=False, **inputs)
    return y


# revision 25
# speedup vs baseline: 1.0023x; 1.0023x over previous
"""Trainium2 Bass kernel for BaseLayerWithLoRA:
    y = x @ W^T + b + (x @ lora_A^T) @ lora_B^T
  x [4,2048,4096] f32, W [4096,4096], b [4096], lora_A [16,4096], lora_B [4096,16]

Sharding: token-parallel across 8 cores (1024 tokens each, full O per core).
No collectives; LoRA is computed per-core on its own token slice.

Per-core device program (bf16 matmuls, fp32 PSUM accumulation):
  phase A (col-tiled 4x): arT partials land at partition offsets 0/32/64/96
    of one PSUM bank per 512-token half.  A zeroing matmul (all-zero
    stationary) opens each bank's accumulation group so the four strips can
    all accumulate with start=False (no whole-bank has_written races), and
    leaves exact 0.0 in the gap partitions.
  lora tail: ONE full 128-row matmul per (o-tile, half) against the
    UNREDUCED strip partials; lora_B^T is replicated at the four 32-row
    offsets inside the weight blob (zeros in gaps), so no cross-partition
    reduction is ever needed and the LDWEIGHTS stays a full-array load
    (prefetchable -- the K=16 row-group load stalled the PE ~190ns/tile).
  wave 1 (first 3 o-tiles): kc-outer over 6 PSUM tiles so the PE rides the
    incoming x-chunk DMAs with no head-of-line blocking; those 3 blobs are
    DMA'd in 5 column pieces from a piece-major contiguous copy (every
    dma_start costs ~0.6us of serial Sync-engine descriptor issue, so each
    piece is ONE full-rate transfer); phase-A strip blocks pad the riding
    gaps.  HAM warmup: zero-accumulating matmuls fill the pre-data idle so
    the PE clock gate is already at K=8/8 when real data arrives.
  waves 2+: o-tile-serial, kc-inner: 33 accumulating matmuls per (ot,half)
    into one PSUM bank; bias fused into the PSUM->SBUF eviction (DVE
    tensor_scalar_add with a per-partition scalar operand).
Host does data layout only (transposes / tiling / packing), no arithmetic.

Built on bacc.Bacc so compile() runs move_matmul_waits_to_ldweights and
generate_event_semaphores.
"""

import sys

if "/opt/trn_rl_repo" not in sys.path:
    sys.path.insert(0, "/opt/trn_rl_repo")

import numpy as np

B, S, I, O, R = 4, 2048, 4096, 4096, 16
NCORES = 8
NTOK = B * S                 # 8192 tokens
TPC = NTOK // NCORES         # 1024 tokens per core


def build_nc(tpc=TPC, i_dim=I, o_dim=O, r=R, tok_tile=512, mm_dtype="bfloat16",
             wave1_ots=3, w_bufs=5, ps_bufs=6, o_bufs=3):
    import concourse.bacc as bacc
    import concourse.mybir as mybir
    import concourse.tile as tile

    KC = i_dim // 128        # contraction chunks
    OT = o_dim // 128        # output-row tiles
    TT = tpc // tok_tile     # token tiles (halves)
    WF = KC * 128 + 128      # per-o-tile weight blob free size (W + lora part)
    NG = KC // 4             # phase-A chunk groups (4 strips each)
    W1 = wave1_ots
    f32 = mybir.dt.float32
    fmm = getattr(mybir.dt, mm_dtype)

    nc = bacc.Bacc("TRN2", target_bir_lowering=False, debug=False)
    xt = nc.declare_dram_parameter("xt", [KC, 128, tpc], fmm, isOutput=False)
    wt = nc.declare_dram_parameter("wt", [OT, 128, WF], fmm, isOutput=False)
    # wave-1 blobs again, piece-major and contiguous per partition so each
    # column piece is ONE full-HBM-rate DMA (cols [3*a:3*b] = piece [a:b]
    # of all W1 blobs, laid out blob-major within the piece)
    wt1 = nc.declare_dram_parameter("wt1", [128, W1 * WF], fmm,
                                    isOutput=False)
    at = nc.declare_dram_parameter("at", [128, KC, r], fmm, isOutput=False)
    bias = nc.declare_dram_parameter("bias", [128, OT], f32, isOutput=False)
    out = nc.declare_dram_parameter("out", [OT, 128, tpc], f32, isOutput=True)

    with tile.TileContext(nc) as tc:
        with (
            tc.tile_pool(name="const", bufs=1) as constp,
            tc.tile_pool(name="xpool", bufs=KC) as xpool,
            tc.tile_pool(name="wpool", bufs=w_bufs) as wpool,
            tc.tile_pool(name="opool", bufs=o_bufs) as opool,
            tc.tile_pool(name="psum", bufs=ps_bufs, space="PSUM") as psum_pool,
        ):
            zeros_sb = constp.tile([128, 512], fmm, name="zeros_sb")
            nc.gpsimd.memset(zeros_sb[:], 0)
            at_sb = constp.tile([128, KC, r], fmm, name="at_sb")
            arT_sb = constp.tile([128, tpc], fmm, name="arT_sb")

            # DMA issue order == completion order (single HW dynamic queue):
            # interleave x chunks with just-in-time wave-1 blob pieces so the
            # PE goes dense as early as possible and never starves.
            xts = [None] * KC

            def dma_x(kc):
                x_t = xpool.tile([128, tpc], fmm, tag="xchunk",
                                 name=f"xchunk{kc}")
                nc.sync.dma_start(x_t[:], xt[kc])
                xts[kc] = x_t

            # all W1 wave-1 blobs live in ONE tile so each column piece is a
            # single 3D DMA: every dma_start costs ~0.6us of serial Sync-
            # engine descriptor issue, so fewer+bigger transfers win the head
            w1_sb = wpool.tile([128, W1, WF], fmm, tag="w1all", bufs=1,
                               name="w1all")
            # wave-1 blob pieces: kc0-1 | kc2-7 | kc8-15 | kc16-23 | kc24-end
            pieces = [(0, 256), (256, 1024), (1024, 2048), (2048, 3072),
                      (3072, WF)]

            def dma_w1_piece(p):
                a, b_ = pieces[p]
                nc.sync.dma_start(w1_sb[:, :, a:b_],
                                  wt1[:, W1 * a:W1 * b_])

            dma_x(0)
            dma_w1_piece(0)
            dma_x(1)
            dma_w1_piece(1)
            dma_x(2)
            dma_x(3)
            dma_x(4)
            nc.sync.dma_start(at_sb[:], at[:])
            dma_x(5)
            dma_w1_piece(2)
            for kc in range(6, 14):
                dma_x(kc)
            dma_w1_piece(3)
            for kc in range(14, 22):
                dma_x(kc)
            dma_w1_piece(4)
            for kc in range(22, KC):
                dma_x(kc)
            b_sb = constp.tile([128, OT], f32, name="b_sb")
            nc.sync.dma_start(b_sb[:], bias[:])
            # prefetch whole blobs into the remaining fresh ring slots
            pre_sb = {}
            for i in range(W1, min(w_bufs, OT)):
                wsb = wpool.tile([128, WF], fmm, tag="wblob", name=f"wsb{i}")
                nc.sync.dma_start(wsb[:], wt[i])
                pre_sb[i] = wsb

            # phase-A PSUM banks; zero-MM opens the accumulation group and
            # writes exact zeros everywhere (incl. strip gap partitions)
            pa = [
                psum_pool.tile([128, tok_tile], f32, bufs=1, name=f"pa{h}")
                for h in range(TT)
            ]
            # zero-MMs depend only on the memset: they issue before any DMA
            # lands and open each bank's accumulation group with exact zeros
            for h in range(TT):
                nc.tensor.matmul(pa[h][:], zeros_sb[:, 0:128], zeros_sb[:],
                                 start=True, stop=False)
            # HAM warmups: accumulate +0 into the phase-A banks during the
            # otherwise-idle head so the PE clock gate opens (K=8/8) before
            # the first data matmul instead of ~3.4us after it
            for w in range(6):
                nc.tensor.matmul(pa[w % TT][:], zeros_sb[:, 0:128],
                                 zeros_sb[:], start=False, stop=False)

            # wave 1: kc-outer over W1 o-tiles riding the x DMA; each phase-A
            # strip block trails the main block of its chunk group so the
            # col-tiled strips stay adjacent (concurrent) in the PE queue
            ps1 = {}
            for i in range(W1):
                for h in range(TT):
                    ps1[(i, h)] = psum_pool.tile(
                        [128, tok_tile], f32, tag="psm", name=f"ps1_{i}_{h}")
            def main_block(g):
                for kc in range(4 * g, 4 * g + 4):
                    for h in range(TT):
                        for i in range(W1):
                            ts = slice(h * tok_tile, (h + 1) * tok_tile)
                            nc.tensor.matmul(
                                ps1[(i, h)][:],
                                w1_sb[:, i, kc * 128:(kc + 1) * 128],
                                xts[kc][:, ts],
                                start=(kc == 0),
                                stop=(kc == KC - 1),
                            )

            def pha_block(g):
                for h in range(TT):
                    ts = slice(h * tok_tile, (h + 1) * tok_tile)
                    for j in range(4):
                        kc = 4 * g + j
                        last = (g == NG - 1 and j == 3)
                        nc.tensor.matmul(
                            pa[h][32 * j:32 * j + r, :],
                            at_sb[:, kc, :],
                            xts[kc][:, ts],
                            start=False,
                            stop=last,
                            tile_position=(0, 32 * j),
                        )

            # groups 0..NG-3 ride the x DMA with their phase-A strips as
            # padding; main g6 streams while the DVE copies arT out; loras
            # before main g7 so the evictions trail the per-tile stop
            # matmuls (kc31)
            for g in range(NG - 2):
                main_block(g)
                pha_block(g)
            pha_block(NG - 2)
            pha_block(NG - 1)
            main_block(NG - 2)
            for h in range(TT):
                ts = slice(h * tok_tile, (h + 1) * tok_tile)
                nc.vector.tensor_copy(arT_sb[:, ts], pa[h][:])
            for h in range(TT):
                ts = slice(h * tok_tile, (h + 1) * tok_tile)
                for i in range(W1):
                    nc.tensor.matmul(
                        ps1[(i, h)][:],
                        w1_sb[:, i, KC * 128:KC * 128 + 128],
                        arT_sb[:, ts],
                        start=False,
                        stop=False,
                    )
            main_block(NG - 1)
            for h in range(TT):
                ts = slice(h * tok_tile, (h + 1) * tok_tile)
                for i in range(W1):
                    o_sb = opool.tile([128, tok_tile], f32, tag="osb",
                                      name=f"osb_w1_{i}_{h}")
                    nc.vector.tensor_scalar_add(o_sb[:], ps1[(i, h)][:],
                                                b_sb[:, i:i + 1])
                    nc.sync.dma_start(out[i, :, ts], o_sb[:])

            # waves 2+: o-tile-serial (proven 216ns/MM steady state)
            for ot in range(W1, OT):
                if ot in pre_sb:
                    w_sb = pre_sb[ot]
                else:
                    w_sb = wpool.tile([128, WF], fmm, tag="wblob",
                                      name=f"wsb{ot}")
                    nc.sync.dma_start(w_sb[:], wt[ot])
                for h in range(TT):
                    ts = slice(h * tok_tile, (h + 1) * tok_tile)
                    ps = psum_pool.tile([128, tok_tile], f32, tag="psm",
                                        name=f"ps_{ot}_{h}")
                    for kc in range(KC):
                        nc.tensor.matmul(
                            ps[:],
                            w_sb[:, kc * 128:(kc + 1) * 128],
                            xts[kc][:, ts],
                            start=(kc == 0),
                            stop=False,
                        )
                    nc.tensor.matmul(
                        ps[:],
                        w_sb[:, KC * 128:KC * 128 + 128],
                        arT_sb[:, ts],
                        start=False,
                        stop=True,
                    )
                    o_sb = opool.tile([128, tok_tile], f32, tag="osb",
                                      name=f"osb_{ot}_{h}")
                    if ot == OT - 1 and h == TT - 1:
                        # split the final eviction so the last output DMA
                        # starts as early as possible (shorter tail)
                        hw = tok_tile // 2
                        for q in range(2):
                            qs = slice(q * hw, (q + 1) * hw)
                            nc.vector.tensor_scalar_add(
                                o_sb[:, qs], ps[:, qs], b_sb[:, ot:ot + 1])
                            nc.sync.dma_start(
                                out[ot, :, h * tok_tile + q * hw:
                                    h * tok_tile + (q + 1) * hw],
                                o_sb[:, qs])
                    else:
                        nc.vector.tensor_scalar_add(o_sb[:], ps[:],
                                                    b_sb[:, ot:ot + 1])
                        nc.sync.dma_start(out[ot, :, ts], o_sb[:])
    nc.compile()
    return nc


def prep_inputs(x, W, b, lora_A, lora_B, tpc=TPC, ncores=NCORES,
                mm_dtype="bfloat16"):
    """Host-side layout marshalling (layout + dtype cast only)."""
    import ml_dtypes

    np_mm = np.float32 if mm_dtype == "float32r" else np.dtype(ml_dtypes.bfloat16)
    i_dim, o_dim, r = W.shape[1], W.shape[0], lora_A.shape[0]
    ntok = tpc * ncores
    x = np.ascontiguousarray(x, dtype=np.float32).reshape(ntok, i_dim)
    W = np.ascontiguousarray(W, dtype=np.float32)
    b = np.ascontiguousarray(b, dtype=np.float32)
    lora_A = np.ascontiguousarray(lora_A, dtype=np.float32)
    lora_B = np.ascontiguousarray(lora_B, dtype=np.float32)

    KC, OT = i_dim // 128, o_dim // 128
    WF = KC * 128 + 128
    # wt blob per o-tile: [ki, kc*128+oo] = W[ot*128+oo, kc*128+ki],
    # last 128 cols: lora_B^T slice replicated at row offsets 0/32/64/96
    # (gap rows zero) to match the unreduced phase-A strip partials.
    wtb = np.zeros((OT, 128, WF), dtype=np_mm)
    wtb[:, :, : KC * 128] = (
        W.reshape(OT, 128, KC, 128).transpose(0, 3, 2, 1).reshape(OT, 128, KC * 128)
    ).astype(np_mm)
    lbT = lora_B.reshape(OT, 128, r).transpose(0, 2, 1).astype(np_mm)
    for g in range(4):
        wtb[:, 32 * g:32 * g + r, KC * 128:] = lbT
    # wave-1 blobs for the combined piece DMAs: piece-major, blob-major
    # within each piece, contiguous per partition
    w1pieces = [(0, 256), (256, 1024), (1024, 2048), (2048, 3072),
                (3072, WF)]
    wt1 = np.ascontiguousarray(np.concatenate(
        [wtb[0:3, :, a:b].transpose(1, 0, 2).reshape(128, -1)
         for (a, b) in w1pieces], axis=1))
    # at[ki, kc, r] = lora_A[r, kc*128+ki]
    at = np.ascontiguousarray(
        lora_A.T.reshape(KC, 128, r).transpose(1, 0, 2).astype(np_mm)
    )
    # bias[p, ot] = b[ot*128+p]
    bias = np.ascontiguousarray(b.reshape(OT, 128).T)

    in_maps = []
    for c in range(ncores):
        xc = x[c * tpc : (c + 1) * tpc]  # [tpc, i_dim]
        # xt[kc, ki, t] = xc[t, kc*128+ki]
        xtc = np.ascontiguousarray(
            xc.reshape(tpc, KC, 128).transpose(1, 2, 0).astype(np_mm)
        )
        in_maps.append({"xt": xtc, "wt": wtb, "wt1": wt1, "at": at,
                        "bias": bias})
    return in_maps


def assemble_output(results):
    # each core: out[OT, 128, tpc] == y_c^T; tokens are block-sharded
    outT = np.concatenate([r["out"] for r in results], axis=2)  # [OT,128,ntok]
    o_dim = outT.shape[0] * 128
    ntok = outT.shape[2]
    y = outT.reshape(o_dim, ntok).T  # [ntok, o_dim]
    return np.ascontiguousarray(y)


def run(trace=False, trace_kwargs=None, mm_dtype="bfloat16", **inputs):
    from concourse.bass_utils import run_bass_kernel_spmd

    nc = build_nc(mm_dtype=mm_dtype)
    in_maps = prep_inputs(mm_dtype=mm_dtype, **inputs)
    res = run_bass_kernel_spmd(
        nc,
        in_maps,
        list(range(NCORES)),
        trace=trace,
        trace_kwargs=trace_kwargs or {},
    )
    return assemble_output(res.results).reshape(B, S, O), res


def kernel(**inputs):
    y, _ = run(trace=False, **inputs)
    return y
